# revision 18
# baseline (speedup 1.0000x reference)
"""Distributed WeightedHGTConv kernel for 8 Trainium2 NeuronCores (Bass/Tile).

Strategy (node/dst-range sharding, self-contained):
  * Nodes are range-sharded across the 8 cores by destination id; every edge
    lives on the core that owns its dst node, so the segment softmax and the
    scatter-add are core-local (no softmax-stat all-reduce needed).
  * Within a core, nodes are SORTED BY TYPE (uniform tiles-per-type across
    cores so one compiled kernel serves all 8): the per-type Q/K/V projection
    becomes ONE matmul per 128-node tile instead of 4 masked ones, and the
    masked-x table shrinks 4x.
  * Host side: edges are sorted by (type-sorted) dst and greedy-packed into
    128-edge tiles with at most 32 segments per tile and no node's edge list
    spanning a tile.  Per tile, a one-hot [edge, 32] matrix turns the segment
    sums of exp-scores (den) and exp*V (num) into one TensorE matmul; four
    tiles' 32-row outputs stack into one 128-partition PSUM slab that is
    scattered straight to a per-node DRAM accumulator (collision-free).
  * The per-edge relation/sign tables (24 distinct rows) are NOT expanded on
    the host: a host-packed one-hot [24, E] fp16 feeds per-tile TensorE
    matmuls against the 24x256 table, and the ScalarE copies the PSUM result
    to fp16 for the DVE multiplies.  This removes the 26MB/core per-edge
    table stream of the previous version.
  * Device phases: (A) per-type projections (1 matmul + bias matmul per
    tile), K|V written as one fp16 table; (AG) one AllGather replicates the
    K|V table; (B) per 16-tile megatile: indirect-gather K|V rows by src and
    Q rows by dst, table expansion, fused score reduce + exp (no
    max-subtraction: scores are bounded), one-hot segment matmul, packed
    scatter; (D) softmax divide + skip gate + layernorm over own nodes.
"""
import sys

sys.path.insert(0, "/opt/trn_rl_repo")

import numpy as np

import os

BATCH_DMA = os.environ.get("BATCH_DMA", "0") == "1"
V8 = os.environ.get("V8", "0") == "1"
CORES = 8
N_NODES = 50000
D = 128
H, DK = 8, 16
T, R = 4, 8
P = 128
MEGA = 12
SEGCAP = 32

_NC_CACHE = {}
_NC_KEYS = ("np_nodes", "t_tiles", "mega")


# --------------------------------------------------------------------------
# host-side preparation
# --------------------------------------------------------------------------
def _host_prep(inputs, n, cores, mega):
    nc_nodes = n // cores

    x = np.asarray(inputs["node_inp"], np.float32)
    nt = np.asarray(inputs["node_type"]).astype(np.int64)
    src = np.asarray(inputs["edge_index"][0]).astype(np.int64)
    dst = np.asarray(inputs["edge_index"][1]).astype(np.int64)
    et = np.asarray(inputs["edge_type"]).astype(np.int64)
    es = np.asarray(inputs["edge_sign"]).astype(np.int64)

    sidx = np.where(es == -1, 0, np.where(es == 1, 1, 2)).astype(np.int64)
    cmb = (et * 3 + sidx).astype(np.int64)

    ones = np.ones((H, DK), np.float32)
    sk_all = np.stack([-ones, ones,
                       np.asarray(inputs["sign_k_neutral"], np.float32)], 0)
    sv_all = np.stack([-ones, ones,
                       np.asarray(inputs["sign_v_neutral"], np.float32)], 0)
    rel_q = np.asarray(inputs["rel_q"], np.float32)
    rel_k = np.asarray(inputs["rel_k"], np.float32)
    rel_v = np.asarray(inputs["rel_v"], np.float32)
    W2tab = (rel_q[:, None] * rel_k[:, None] * sk_all[None]).reshape(R * 3, D)
    Wvtab = (rel_v[:, None] * sv_all[None]).reshape(R * 3, D)
    W24 = np.concatenate([W2tab, Wvtab], 1).astype(np.float16)  # [24, 256]
    bias4 = 4.0 * np.asarray(inputs["rel_bias"], np.float32)

    alpha = 1.0 / (1.0 + np.exp(-np.asarray(inputs["skip"], np.float32)))
    ln_g = np.asarray(inputs["ln_gamma"], np.float32)
    ln_b = np.asarray(inputs["ln_beta"], np.float32)

    # ---- per-core type sort with uniform tiles-per-type ----
    tcounts = np.zeros((cores, T), np.int64)
    orders = []
    for c in range(cores):
        nt_c = nt[c * nc_nodes:(c + 1) * nc_nodes]
        order = np.argsort(nt_c, kind="stable")  # sorted pos -> orig local
        orders.append(order)
        for t in range(T):
            tcounts[c, t] = int((nt_c == t).sum())
    tpt = int(np.ceil(tcounts.max() / P))           # tiles per type
    ntn = T * tpt
    np_nodes = ntn * P

    # slot (type-sorted padded position) for each local node
    slots = []
    for c in range(cores):
        nt_c = nt[c * nc_nodes:(c + 1) * nc_nodes]
        slot = np.empty(nc_nodes, np.int64)
        order = orders[c]
        pos = 0
        for t in range(T):
            ct = tcounts[c, t]
            slot[order[pos:pos + ct]] = t * tpt * P + np.arange(ct)
            pos += ct
        slots.append(slot)

    # global kv_all row for any source node
    kvrow = np.empty(n, np.int64)
    for c in range(cores):
        kvrow[c * nc_nodes:(c + 1) * nc_nodes] = c * np_nodes + slots[c]

    # ---- edge tiling per core ----
    order_e = np.argsort(dst, kind="stable")
    dsts = dst[order_e]
    srcs = src[order_e]
    cmbs = cmb[order_e]
    ets = et[order_e]

    core_lo = np.searchsorted(dsts, np.arange(cores) * nc_nodes)
    core_hi = np.searchsorted(dsts, (np.arange(cores) + 1) * nc_nodes)

    per_core_tiles = []
    per_core_edata = []
    tile_counts = []
    for c in range(cores):
        lo, hi = core_lo[c], core_hi[c]
        d_slot = slots[c][(dsts[lo:hi] - c * nc_nodes)]
        o2 = np.argsort(d_slot, kind="stable")
        d_slot = d_slot[o2]
        e_src = srcs[lo:hi][o2]
        e_cmb = cmbs[lo:hi][o2]
        e_et = ets[lo:hi][o2]
        nodes, starts, counts = np.unique(d_slot, return_index=True,
                                          return_counts=True)
        tiles = []
        cur = []
        fill = 0
        for nid, st, ct in zip(nodes, starts, counts):
            assert ct <= P, f"node degree {ct} > {P}"
            if fill + ct > P or len(cur) >= SEGCAP:
                tiles.append(cur)
                cur = []
                fill = 0
            cur.append((int(nid), int(st), int(ct)))
            fill += ct
        if cur:
            tiles.append(cur)
        per_core_tiles.append(tiles)
        per_core_edata.append((e_src, e_cmb, e_et))
        tile_counts.append(len(tiles))

    t_tiles = int(np.ceil(max(tile_counts) / mega) * mega)
    t4 = t_tiles // 3

    pc = []
    for c in range(cores):
        e_src, e_cmb, e_et = per_core_edata[c]

        idx2 = np.zeros((t_tiles, P), np.int32)
        seg_e = np.full((t_tiles, P), 127, np.int16)
        oh24 = np.zeros((24, t_tiles, P), np.float16)
        b4_e = np.zeros((t_tiles, P, H), np.float16)
        # default scatter target: per-partition dump row (zones past np_nodes)
        sci = np.empty((t4, 96), np.int32)
        sci[:] = np_nodes + np.arange(96)[None, :]

        spare = []  # (tile, seg) slots whose matmul rows are guaranteed zero
        covered = np.zeros(np_nodes, bool)
        tiles = per_core_tiles[c]
        for ti, tl in enumerate(tiles):
            ep = 0
            for si, (nid, st, ct) in enumerate(tl):
                sl = slice(st, st + ct)
                rows = slice(ep, ep + ct)
                idx2[ti, rows] = kvrow[e_src[sl]] * 8192 + nid
                seg_e[ti, rows] = si
                oh24[e_cmb[sl], ti, np.arange(ep, ep + ct)] = 1.0
                b4_e[ti, rows] = bias4[e_et[sl]]
                sci[ti // 3, 32 * (ti % 3) + si] = nid
                covered[nid] = True
                ep += ct
            for si in range(len(tl), SEGCAP):
                spare.append((ti, si))
        for ti in range(len(tiles), t_tiles):
            for si in range(SEGCAP):
                spare.append((ti, si))

        # zero-fill uncovered accumulator rows (isolated + type-pad nodes)
        need = np.nonzero(~covered)[0]
        assert len(need) <= len(spare), (len(need), len(spare))
        for row, (ti, si) in zip(need, spare):
            sci[ti // 3, 32 * (ti % 3) + si] = row

        # ---- phase A tensors ----
        x_slot = np.zeros((np_nodes, D), np.float32)
        nt_slot = np.zeros(np_nodes, np.int64)
        nt_slot[:] = np.arange(np_nodes) // (tpt * P)
        x_slot[slots[c]] = x[c * nc_nodes:(c + 1) * nc_nodes]
        valid = np.zeros(np_nodes, bool)
        valid[slots[c]] = True
        xmT = np.ascontiguousarray(
            x_slot.reshape(ntn, P, D).transpose(0, 2, 1)).astype(np.float16)

        # ---- phase D tensors ----
        a_n = alpha[nt_slot].astype(np.float32)[:, None]
        a_n[~valid] = 1.0
        x1a = ((1.0 - a_n) * x_slot).astype(np.float16)

        pc.append(dict(
            idx2=idx2, seg_e=seg_e, oh24=oh24, b4_e=b4_e, sci=sci,
            xmT=xmT, x1a=x1a, alpha_n=a_n,
            slots=slots[c],
        ))

    shared = dict(
        Wqkv=np.stack([np.concatenate(
            [np.asarray(inputs["Wq"], np.float32)[t],
             np.asarray(inputs["Wk"], np.float32)[t],
             np.asarray(inputs["Wv"], np.float32)[t]], axis=1)
            for t in range(T)]).astype(np.float16),
        bqkv=np.stack([np.concatenate(
            [np.asarray(inputs["bq"], np.float32)[t],
             np.asarray(inputs["bk"], np.float32)[t],
             np.asarray(inputs["bv"], np.float32)[t]])
            for t in range(T)]).astype(np.float16),
        W24=W24,
        lngb=np.concatenate([ln_g, ln_b], 1).astype(np.float16),
    )
    meta = dict(t_tiles=t_tiles, mega=mega, cores=cores,
                nc_nodes=nc_nodes, np_nodes=np_nodes, ntn=ntn)
    return pc, shared, meta


# --------------------------------------------------------------------------
# device kernel
# --------------------------------------------------------------------------
def _build_nc(np_nodes, t_tiles, mega, cores, repeat=1, skip_collective=False):
    import concourse.bass as bass
    import concourse.tile as tile
    from concourse import mybir, bacc

    F16 = mybir.dt.float16
    BF16 = mybir.dt.bfloat16
    F8 = mybir.dt.float8e4
    F32 = mybir.dt.float32
    I32 = mybir.dt.int32
    I16 = mybir.dt.int16
    KVW = 3 * D if V8 else 4 * D  # kv row bytes (K f16 | V f8-or-f16)

    ntn = np_nodes // P
    tpt = ntn // T
    nmega = t_tiles // mega
    t4 = t_tiles // 3
    hm = mega // 2

    nc = bacc.Bacc()
    dp = nc.declare_dram_parameter

    xmT = dp("xmT", [ntn, D, P], F16, isOutput=False)
    Wqkv = dp("Wqkv", [T, D, 3 * D], F16, isOutput=False)
    bqkv = dp("bqkv", [T, 3 * D], F16, isOutput=False)
    W24 = dp("W24", [24, 2 * D], F16, isOutput=False)
    lngb = dp("lngb", [T, 2 * D], F16, isOutput=False)
    idx2 = dp("idx2", [t_tiles, P], I32, isOutput=False)
    seg_e = dp("seg_e", [t_tiles, P], I16, isOutput=False)
    oh24 = dp("oh24", [24, t_tiles, P], F16, isOutput=False)
    b4_e = dp("b4_e", [t_tiles, P, H], F16, isOutput=False)
    sci = dp("sci", [t4, 96], I32, isOutput=False)
    x1a = dp("x1a", [np_nodes, D], F16, isOutput=False)
    alpha_n = dp("alpha_n", [np_nodes, 1], F32, isOutput=False)

    out = dp("out", [np_nodes, D], F32, isOutput=True)

    q_loc = nc.dram_tensor("q_loc", [np_nodes, D], F16)
    kv_own = nc.dram_tensor("kv_own", [np_nodes, KVW], F8)
    kv_all = nc.dram_tensor("kv_all", [cores * np_nodes, KVW], F8,
                            addr_space="Shared")
    acc = nc.dram_tensor("acc", [np_nodes + P, 8 + D], F32)

    with tile.TileContext(nc) as tc:
        with tc.tile_pool(name="sb", bufs=2) as sb, \
             tc.tile_pool(name="sbc", bufs=1) as sbc:
          for _rep in range(repeat):

            # ---- Phase A: per-type projections (type-sorted tiles) ----
            wq_t = [sbc.tile([D, 3 * D], F16, tag=f"wq{t}", name=f"wq{t}")
                    for t in range(T)]
            for t in range(T):
                nc.sync.dma_start(out=wq_t[t][:], in_=Wqkv[t])
            bq_t = [sbc.tile([1, 3 * D], F16, tag=f"bq{t}", name=f"bq{t}")
                    for t in range(T)]
            for t in range(T):
                nc.sync.dma_start(out=bq_t[t][:], in_=bqkv[t:t + 1])
            ones1 = sbc.tile([1, P], F16, tag="ones1")
            nc.vector.memset(ones1[:], 1.0)

            with tc.tile_pool(name="psA", bufs=2, space="PSUM") as psA:
                for i in range(ntn):
                    t = i // tpt
                    xm = sb.tile([D, P], F16, tag="xm")
                    nc.sync.dma_start(out=xm[:], in_=xmT[i])
                    ps = psA.tile([P, 3 * D], F32, tag="psA")
                    nc.tensor.matmul(ps[:], lhsT=xm[:], rhs=wq_t[t][:],
                                     start=True, stop=False)
                    nc.tensor.matmul(ps[:], lhsT=ones1[:], rhs=bq_t[t][:],
                                     start=False, stop=True)
                    qo = sb.tile([P, D], F16, tag="qo")
                    nc.scalar.copy(out=qo[:], in_=ps[:, 0:D])
                    nc.sync.dma_start(out=q_loc[i * P:(i + 1) * P], in_=qo[:])
                    rs = slice(i * P, (i + 1) * P)
                    if V8:
                        kvo = sb.tile([P, D], F16, tag="kvo")
                        nc.vector.tensor_copy(out=kvo[:], in_=ps[:, D:2 * D])
                        kvv = sb.tile([P, D], F8, tag="kvv")
                        nc.vector.tensor_copy(out=kvv[:], in_=ps[:, 2 * D:3 * D])
                        nc.sync.dma_start(
                            out=kv_own[rs, 0:2 * D].bitcast(F16), in_=kvo[:])
                        nc.sync.dma_start(out=kv_own[rs, 2 * D:3 * D],
                                          in_=kvv[:])
                    else:
                        kvo = sb.tile([P, 2 * D], F16, tag="kvo")
                        nc.vector.tensor_copy(out=kvo[:], in_=ps[:, D:3 * D])
                        nc.sync.dma_start(out=kv_own[rs].bitcast(F16),
                                          in_=kvo[:])

            # ---- replicate the K|V table ----
            if not skip_collective:
                nc.gpsimd.collective_compute(
                    "AllGather", mybir.AluOpType.bypass,
                    replica_groups=[list(range(cores))],
                    ins=[kv_own[:]],
                    outs=[kv_all[:]],
                )

            iv32 = sbc.tile([P, SEGCAP], I16, tag="iv32")
            nc.gpsimd.iota(iv32[:], pattern=[[1, SEGCAP]], base=0,
                           channel_multiplier=0)
            w24t = sbc.tile([24, 2 * D], F16, tag="w24t")
            nc.sync.dma_start(out=w24t[:], in_=W24[:])
            gb_t = [sbc.tile([1, 2 * D], F16, tag=f"gb{t}", name=f"gb{t}")
                    for t in range(T)]
            for t in range(T):
                nc.sync.dma_start(out=gb_t[t][:], in_=lngb[t:t + 1])

            # ---- Phase B: edge megatiles ----
            with tc.tile_pool(name="psE", bufs=1, space="PSUM") as psEp, \
                 tc.tile_pool(name="psS", bufs=2, space="PSUM") as psSp:
                for m in range(nmega):
                    t0 = m * mega
                    ix = sb.tile([P, mega], I32, tag="ix", bufs=3)
                    nc.sync.dma_start(
                        out=ix[:],
                        in_=idx2[t0:t0 + mega].rearrange("t p -> p t"))
                    kvi = sb.tile([P, mega, 1], I32, tag="kvi")
                    nc.vector.tensor_scalar(
                        out=kvi[:, :, 0], in0=ix[:], scalar1=13, scalar2=None,
                        op0=mybir.AluOpType.logical_shift_right)
                    qi = sb.tile([P, mega, 1], I32, tag="qi")
                    nc.vector.tensor_scalar(
                        out=qi[:, :, 0], in0=ix[:], scalar1=8191, scalar2=None,
                        op0=mybir.AluOpType.bitwise_and)
                    segt = sb.tile([P, mega], I16, tag="segt", bufs=3)
                    nc.sync.dma_start(
                        out=segt[:],
                        in_=seg_e[t0:t0 + mega].rearrange("t p -> p t"))
                    o24 = sb.tile([24, mega, P], F16, tag="o24", bufs=3)
                    nc.sync.dma_start(out=o24[:], in_=oh24[:, t0:t0 + mega])
                    b4 = sb.tile([P, mega, H], F16, tag="b4", bufs=3)
                    nc.sync.dma_start(
                        out=b4[:],
                        in_=b4_e[t0:t0 + mega].rearrange("t p c -> p t c"))
                    scit = sb.tile([96, 4, 1], I32, tag="scit", bufs=3)
                    nc.sync.dma_start(
                        out=scit[:, :, 0],
                        in_=sci[m * 4:m * 4 + 4].rearrange("q p -> p q"))

                    oh = sb.tile([P, mega, SEGCAP], BF16, tag="oh")
                    nc.vector.tensor_tensor(
                        out=oh[:],
                        in0=iv32[:, None, :].to_broadcast([P, mega, SEGCAP]),
                        in1=segt[:, :, None].to_broadcast([P, mega, SEGCAP]),
                        op=mybir.AluOpType.is_equal)

                    kvg = sb.tile([P, mega, KVW], F8, tag="kvg")
                    qg = sb.tile([P, mega, D], F16, tag="qg")
                    if BATCH_DMA:
                        nc.gpsimd.indirect_dma_start(
                            out=kvg[:], out_offset=None,
                            in_=kv_all[:],
                            in_offset=bass.IndirectOffsetOnAxis(
                                ap=kvi[:], axis=0))
                        nc.gpsimd.indirect_dma_start(
                            out=qg[:], out_offset=None,
                            in_=q_loc[:],
                            in_offset=bass.IndirectOffsetOnAxis(
                                ap=qi[:], axis=0))
                    else:
                        for g in range(mega):
                            nc.gpsimd.indirect_dma_start(
                                out=kvg[:, g], out_offset=None,
                                in_=kv_all[:],
                                in_offset=bass.IndirectOffsetOnAxis(
                                    ap=kvi[:, g], axis=0))
                            nc.gpsimd.indirect_dma_start(
                                out=qg[:, g], out_offset=None,
                                in_=q_loc[:],
                                in_offset=bass.IndirectOffsetOnAxis(
                                    ap=qi[:, g], axis=0))

                    osc = sb.tile([96, 4, 8 + D], F32, tag="osc")
                    for h in range(2):
                        gs = slice(h * hm, (h + 1) * hm)
                        psE = psEp.tile([P, hm, 2 * D], F32, tag="psE")
                        for g in range(hm):
                            nc.tensor.matmul(
                                psE[:, g, :], lhsT=o24[:, h * hm + g, :],
                                rhs=w24t[:], start=True, stop=True)
                        wws = sb.tile([P, hm, 2 * D], F16, tag="wws")
                        nc.scalar.copy(out=wws[:], in_=psE[:])

                        q2 = sb.tile([P, hm, D], F16, tag="q2")
                        nc.vector.tensor_tensor(
                            out=q2[:], in0=qg[:, gs], in1=wws[:, :, 0:D],
                            op=mybir.AluOpType.mult)
                        sprod = sb.tile([P, hm, D], F16, tag="sprod")
                        nc.vector.tensor_tensor(
                            out=sprod[:], in0=q2[:],
                            in1=kvg[:, gs, 0:2 * D].bitcast(F16),
                            op=mybir.AluOpType.mult)
                        sred = sb.tile([P, hm, H], F32, tag="sred")
                        nc.vector.reduce_sum(
                            out=sred[:],
                            in_=sprod[:].rearrange("p m (h k) -> p (m h) k",
                                                   k=DK),
                            axis=mybir.AxisListType.X)
                        s3 = sb.tile([P, hm, H], F32, tag="s3")
                        nc.vector.tensor_tensor(out=s3[:], in0=sred[:],
                                                in1=b4[:, gs],
                                                op=mybir.AluOpType.add)
                        rt = sb.tile([P, hm, 8 + D], BF16, tag="rt")
                        nc.scalar.activation(
                            out=rt[:, :, 0:8], in_=s3[:],
                            func=mybir.ActivationFunctionType.Exp, scale=0.25)
                        v2 = sb.tile([P, hm, D], F16, tag="v2")
                        if V8:
                            nc.vector.tensor_tensor(
                                out=v2[:], in0=kvg[:, gs, 2 * D:3 * D],
                                in1=wws[:, :, D:2 * D],
                                op=mybir.AluOpType.mult)
                        else:
                            nc.vector.tensor_tensor(
                                out=v2[:],
                                in0=kvg[:, gs, 2 * D:4 * D].bitcast(F16),
                                in1=wws[:, :, D:2 * D],
                                op=mybir.AluOpType.mult)
                        nc.vector.tensor_tensor(
                            out=rt[:, :, 8:8 + D].rearrange(
                                "p m (h k) -> p m h k", k=DK),
                            in0=v2[:].rearrange("p m (h k) -> p m h k", k=DK),
                            in1=rt[:, :, 0:8, None].to_broadcast(
                                [P, hm, 8, DK]),
                            op=mybir.AluOpType.mult)

                        psS = psSp.tile([96, 2, 256], F32, tag="psS")
                        for g in range(hm):
                            tg = h * hm + g
                            p0 = 32 * (tg % 3)
                            nc.tensor.matmul(
                                psS[p0:p0 + 32, g // 3, 0:8 + D],
                                lhsT=oh[:, tg, :], rhs=rt[:, g, :],
                                start=True, stop=True)
                        nc.vector.tensor_copy(
                            out=osc[:, 2 * h:2 * h + 2, :],
                            in_=psS[:, :, 0:8 + D])
                    if BATCH_DMA:
                        nc.gpsimd.indirect_dma_start(
                            out=acc[:], out_offset=bass.IndirectOffsetOnAxis(
                                ap=scit[:, :, 0:1], axis=0),
                            in_=osc[:], in_offset=None)
                    else:
                        for q in range(4):
                            nc.gpsimd.indirect_dma_start(
                                out=acc[:],
                                out_offset=bass.IndirectOffsetOnAxis(
                                    ap=scit[:, q, 0:1], axis=0),
                                in_=osc[:, q, :], in_offset=None)

            # ---- Phase D: softmax divide + skip gate + layernorm ----
            with tc.tile_pool(name="psD", bufs=2, space="PSUM") as psDp:
              for i in range(ntn):
                t = i // tpt
                rs = slice(i * P, (i + 1) * P)
                ac = sb.tile([P, 8 + D], F32, tag="ac")
                nc.sync.dma_start(out=ac[:], in_=acc[rs])
                xa = sb.tile([P, D], F16, tag="xa")
                nc.sync.dma_start(out=xa[:], in_=x1a[rs])
                gb = psDp.tile([P, 2 * D], F32, tag="gb")
                nc.tensor.matmul(gb[:], lhsT=ones1[:], rhs=gb_t[t][:],
                                 start=True, stop=True)
                al = sb.tile([P, 1], F32, tag="al")
                nc.sync.dma_start(out=al[:], in_=alpha_n[rs])

                rec = sb.tile([P, H], F32, tag="rec")
                nc.vector.tensor_scalar_add(rec[:], ac[:, 0:8], 1e-16)
                rec2 = sb.tile([P, H], F32, tag="rec2")
                nc.vector.reciprocal(rec2[:], rec[:])
                rec3 = sb.tile([P, H], F32, tag="rec3")
                nc.vector.tensor_scalar_mul(rec3[:], rec2[:], al[:, 0:1])
                o1 = sb.tile([P, D], F32, tag="o1")
                nc.vector.tensor_tensor(
                    out=o1[:].rearrange("p (h k) -> p h k", k=DK),
                    in0=ac[:, 8:8 + D].rearrange("p (h k) -> p h k", k=DK),
                    in1=rec3[:, :, None].to_broadcast([P, H, DK]),
                    op=mybir.AluOpType.mult)
                pre = sb.tile([P, D], F32, tag="pre")
                nc.vector.tensor_tensor(out=pre[:], in0=o1[:], in1=xa[:],
                                        op=mybir.AluOpType.add)
                ssum = sb.tile([P, 1], F32, tag="ssum")
                cpy = sb.tile([P, D], F32, tag="cpy")
                nc.scalar.activation(
                    out=cpy[:], in_=pre[:],
                    func=mybir.ActivationFunctionType.Identity,
                    bias=0.0, accum_out=ssum[:])
                nmu = sb.tile([P, 1], F32, tag="nmu")
                nc.vector.tensor_scalar_mul(nmu[:], ssum[:], -1.0 / D)
                sq = sb.tile([P, D], F32, tag="sq")
                vsum = sb.tile([P, 1], F32, tag="vsum")
                nc.scalar.activation(
                    out=sq[:], in_=pre[:],
                    func=mybir.ActivationFunctionType.Square,
                    bias=nmu[:, 0:1], accum_out=vsum[:])
                veps = sb.tile([P, 1], F32, tag="veps")
                nc.vector.tensor_scalar(out=veps[:], in0=vsum[:],
                                        scalar1=1.0 / D, scalar2=1e-5,
                                        op0=mybir.AluOpType.mult,
                                        op1=mybir.AluOpType.add)
                sd = sb.tile([P, 1], F32, tag="sd")
                nc.scalar.activation(out=sd[:], in_=veps[:],
                                     func=mybir.ActivationFunctionType.Sqrt)
                rstd = sb.tile([P, 1], F32, tag="rstd")
                nc.vector.reciprocal(rstd[:], sd[:])
                d2g = sb.tile([P, D], F32, tag="d2g")
                nc.vector.scalar_tensor_tensor(
                    out=d2g[:], in0=pre[:], scalar=nmu[:, 0:1],
                    in1=gb[:, 0:D], op0=mybir.AluOpType.add,
                    op1=mybir.AluOpType.mult)
                of2 = sb.tile([P, D], F32, tag="of2")
                nc.vector.scalar_tensor_tensor(
                    out=of2[:], in0=d2g[:], scalar=rstd[:, 0:1],
                    in1=gb[:, D:2 * D], op0=mybir.AluOpType.mult,
                    op1=mybir.AluOpType.add)
                nc.sync.dma_start(out=out[rs], in_=of2[:])

    nc.compile()
    return nc


def _in_map_for_core(pcd, shared):
    m = dict(shared)
    for k in ("idx2", "seg_e", "oh24", "b4_e", "sci", "xmT",
              "x1a", "alpha_n"):
        m[k] = pcd[k]
    return m


# --------------------------------------------------------------------------
# entry point
# --------------------------------------------------------------------------
def kernel(**inputs):
    import jax
    # The on-disk XLA compilation cache does not key on the embedded BIR
    # payload of the bass_exec custom call; a stale hit returns a NEFF for a
    # different kernel body.  Always compile fresh.
    try:
        jax.config.update("jax_enable_compilation_cache", False)
    except Exception:
        pass
    from concourse.bass_utils import run_bass_kernel_spmd

    pc, shared, meta = _host_prep(inputs, N_NODES, CORES, MEGA)
    key = (meta["np_nodes"], meta["t_tiles"], meta["mega"])
    if key not in _NC_CACHE:
        _NC_CACHE[key] = _build_nc(*key, CORES)
    nc = _NC_CACHE[key]

    in_maps = [_in_map_for_core(pc[c], shared) for c in range(CORES)]
    res = run_bass_kernel_spmd(nc, in_maps, list(range(CORES)))

    nc_nodes = meta["nc_nodes"]
    out = np.concatenate(
        [res.results[c]["out"][pc[c]["slots"]] for c in range(CORES)], 0)
    return out.astype(np.float32)
